# revision 2
# baseline (speedup 1.0000x reference)
# nn_ClauseRec on one TRN2 chip (8 NeuronCores).
#
# Reference model: 2x SAGEConv(mean) + 1x GraphConv(sum) + linear head, then
# softmax(logits, axis=1) on a [N, 1] tensor. A softmax over a size-1 axis is
# identically 1.0 (exp(z)/exp(z)), so the reference output is the constant
# ones vector for every node, independent of x, the edge list, and all
# weights. The entire 3-layer GNN is dead code with respect to the output.
#
# The kernel is therefore the constant-folded program: each core writes its
# node shard of the output (1.0 per node) with a single DMA — the minimum
# possible device program, since an ExternalOutput lives in DRAM and DRAM is
# only reachable via DMA. Nodes are sharded contiguously across the 8 cores
# (6250 per core); the host concatenates the shards.
#
# The ones are staged as a DRAM input and copied DRAM->DRAM as one fat-row
# [1, 6250] transfer (25 KB, contiguous): one DMA instruction, no engine
# dependency chain, no per-element descriptor blowup (a [6250, 1] access
# pattern would emit one 4-byte descriptor per row).
import functools
import numpy as np

N = 50000
NCORES = 8
SHARD = N // NCORES  # 6250


@functools.lru_cache(maxsize=2)
def _compile(repeat=1):
    # repeat > 1 exists only for slope benchmarking (test.py): the body is
    # repeated inside one NEFF so wall-clock differences isolate device time.
    import concourse.mybir as mybir
    from concourse import bacc, tile

    dt = mybir.dt
    nc = bacc.Bacc("TRN2", target_bir_lowering=False, num_devices=NCORES)
    out_d = nc.dram_tensor("out", [1, SHARD], dt.float32, kind="ExternalOutput")
    ones_d = nc.dram_tensor("ones", [1, SHARD], dt.float32, kind="ExternalInput")

    with tile.TileContext(nc):
        for _ in range(repeat):
            nc.sync.dma_start(out_d.ap(), ones_d.ap())

    nc.compile()
    return nc


def kernel(
    x,
    Wl1,
    Wr1,
    b1,
    Wl2,
    Wr2,
    b2,
    Wrel3,
    Wroot3,
    b3,
    Wlin,
    blin,
    edge_index,
):
    from concourse.bass_utils import run_bass_kernel_spmd

    nc = _compile()
    ones = np.ones((1, SHARD), np.float32)
    in_maps = [{"ones": ones} for _ in range(NCORES)]
    res = run_bass_kernel_spmd(nc, in_maps, list(range(NCORES)))
    out = np.empty((N, 1), dtype=np.float32)
    for k in range(NCORES):
        out[k * SHARD : (k + 1) * SHARD, 0] = res.results[k]["out"][0]
    kernel._res = res
    return out


# revision 6
# speedup vs baseline: 10.8674x; 10.8674x over previous
# nn_ClauseRec on one TRN2 chip (8 NeuronCores).
#
# Reference model: 2x SAGEConv(mean) + 1x GraphConv(sum) + linear head, then
# softmax(logits, axis=1) on a [N, 1] tensor. A softmax over a size-1 axis is
# identically 1.0 (exp(z)/exp(z)), so the reference output is the constant
# ones vector for every node, independent of x, the edge list, and all
# weights. The entire 3-layer GNN is dead code with respect to the output.
#
# The kernel is therefore the constant-folded program: each core writes its
# node shard of the output (1.0 per node) via DMA — the minimum possible
# device program, since an ExternalOutput lives in DRAM and DRAM is only
# reachable via DMA. Nodes are sharded contiguously across the 8 cores
# (6250 per core); the host concatenates the shards.
#
# Span engineering (CoreSim cost model, TRN2):
#  - Raw nc.Block() instead of TileContext (-400 ns of extra barrier).
#  - The unconditional Bass.__init__ all-engine barrier + the Block-exit
#    barrier are suppressed (they only order engines; this program has no
#    cross-engine dependencies), so the DMA trigger issues at t~0 instead of
#    serializing behind the Pool const-memset preamble:    -400 ns.
#  - Output staged as [16, 512] f32 (8192 slots, power-of-2 descriptors;
#    host reads the first 6250) and split into two 16 KB halves issued
#    concurrently on the SP and ACT HWDGE queues, hiding the payload time
#    entirely behind the fixed DMA path (seq + DGE + completion-semaphore
#    propagation):                                          -460 ns.
#  Simulated single-shot span: 2217 ns (vs 3481 ns for the plain
#  TileContext version); the remaining cost is the irreducible fixed path
#  of a single completion-synchronized DMA.
import functools
import numpy as np

N = 50000
NCORES = 8
SHARD = N // NCORES          # 6250
OUT_SHAPE = (16, 512)        # 8192 >= SHARD slots, power-of-2 rows


def _build_fast(repeat=1):
    import concourse.bass as bass_mod
    import concourse.mybir as mybir
    from concourse import bacc

    dt = mybir.dt
    orig_barrier = bass_mod.Bass.all_engine_barrier
    bass_mod.Bass.all_engine_barrier = lambda self, *a, **k: None
    try:
        nc = bacc.Bacc("TRN2", target_bir_lowering=False, num_devices=NCORES)
        out_d = nc.dram_tensor("out", list(OUT_SHAPE), dt.float32, kind="ExternalOutput")
        ones_d = nc.dram_tensor("ones", list(OUT_SHAPE), dt.float32, kind="ExternalInput")
        s1 = nc.alloc_semaphore("s1")
        s2 = nc.alloc_semaphore("s2")
        h = OUT_SHAPE[0] // 2
        # repeat > 1 exists only for slope benchmarking (test.py): each
        # iteration is a complete, completion-synchronized round trip with a
        # cumulative wait threshold. Semaphore values appear to be 16-bit
        # signed on this path (16*2048 = 32768 hung the device; engine-side
        # sem_inc(-16) decrements fail outright), so the threshold must stay
        # below 32768.
        assert 16 * repeat < 32768, "semaphore threshold would overflow int16"
        with nc.Block() as blk:
            def chain(eng, dst, src, sem):
                for i in range(repeat):
                    eng.dma_start(dst, src).then_inc(sem, 16)
                    eng.wait_ge(sem, 16 * (i + 1))

            @blk.sync
            def _(eng):
                chain(eng, out_d[:h], ones_d[:h], s1)

            @blk.scalar
            def _(eng):
                chain(eng, out_d[h:], ones_d[h:], s2)
    finally:
        bass_mod.Bass.all_engine_barrier = orig_barrier
    nc.compile()
    return nc


def _build_safe(repeat=1):
    # Conservative fallback: plain TileContext single-DMA program.
    import concourse.mybir as mybir
    from concourse import bacc, tile

    dt = mybir.dt
    nc = bacc.Bacc("TRN2", target_bir_lowering=False, num_devices=NCORES)
    out_d = nc.dram_tensor("out", list(OUT_SHAPE), dt.float32, kind="ExternalOutput")
    ones_d = nc.dram_tensor("ones", list(OUT_SHAPE), dt.float32, kind="ExternalInput")
    with tile.TileContext(nc):
        for _ in range(repeat):
            nc.sync.dma_start(out_d.ap(), ones_d.ap())
    nc.compile()
    return nc


@functools.lru_cache(maxsize=4)
def _compile(repeat=1, safe=False):
    return (_build_safe if safe else _build_fast)(repeat)


def _run(repeat=1, safe=False):
    from concourse.bass_utils import run_bass_kernel_spmd

    nc = _compile(repeat, safe)
    ones = np.ones(OUT_SHAPE, np.float32)
    in_maps = [{"ones": ones} for _ in range(NCORES)]
    return run_bass_kernel_spmd(nc, in_maps, list(range(NCORES)))


def kernel(
    x,
    Wl1,
    Wr1,
    b1,
    Wl2,
    Wr2,
    b2,
    Wrel3,
    Wroot3,
    b3,
    Wlin,
    blin,
    edge_index,
):
    # Transient runtime errors (axon/NRT flakes) get one fast-path retry;
    # a persistent failure of the barrier-free program falls back to the
    # conservative TileContext variant.
    try:
        res = _run()
    except Exception:
        try:
            res = _run()
        except Exception:
            res = _run(safe=True)
    out = np.empty((N, 1), dtype=np.float32)
    for k in range(NCORES):
        out[k * SHARD : (k + 1) * SHARD, 0] = res.results[k]["out"].reshape(-1)[:SHARD]
    kernel._res = res
    return out
